# revision 24
# baseline (speedup 1.0000x reference)
"""Trainium2 Bass kernel for the Mamba-style selective-scan block
(nn_Block_24962349924931).

Shapes: x [2, 4096, 1024]; D_MODEL=1024, D_INNER=2048, D_STATE=16, K=3.

Sharding: 8 cores = DP2 (batch) x TP4 (d_inner channels, 512/core).
Two device launches with a host-side exchange of u between them:

  Launch A: u = silu(im2col(x) @ W_fused + b_fused)   [per-core channel shard]
            where W_fused = in_w @ conv_w (conv + in_proj fused on host).
  host: gather u shards -> full u per batch.
  Launch B (per-state layout):
    stage 1: delta = softplus(u @ dt_w^T + dt_b)  [channel-major, 128ch x L]
             B/C projections emitted compactly as [32, L] (16 B rows, 16 C).
    stage 2: for each channel f-tile (4) x state n (16):
             Bb/Cb = DMA 0-stride partition-broadcast of B[n]/C[n] to 128 rows;
             dA = exp(A[ch,n] * delta)  (ACT, per-partition scale);
             X = (delta*u) * Bb        (DVE 2x bf16);
             h = tensor_tensor_scan(dA, X) over the whole L in one op;
             psY[tc] += I @ (h * Cb)   (PE accumulates the 16 states in PSUM).
    stage 3: y_total = psY + u*Dskip; out_partial = y_total @ out_w^T.
  host: sum the 4 TP partials per batch, add out_b, stack batches.
"""
import sys
sys.path.insert(0, "/opt/trn_rl_repo")

import numpy as np
import ml_dtypes

import concourse.bacc as bacc
import concourse.mybir as mybir
from concourse import bass_utils
from concourse.tile import TileContext

F32 = mybir.dt.float32
BF16 = mybir.dt.bfloat16
AL = mybir.AluOpType
AF = mybir.ActivationFunctionType
BF16NP = ml_dtypes.bfloat16

# ---- problem constants ----
B, L, D, E, N, K = 2, 4096, 1024, 2048, 16, 3
NCORES = 8
TPG = 4              # cores per batch (tensor parallel group)
EL = E // TPG        # 512 channels per core
FT = EL // 128       # 4 f-tiles per core
KT_U = E // 128      # 16 k-tiles over full E
KT_X = (D * K) // 128  # 24 k-tiles over im2col contraction
TC = L // 512        # 8 t-chunks

# timing loop reps (0 = single shot); test.py rebuilds with reps>0
_LOOP_REPS = 0


def _bf16(a):
    return np.ascontiguousarray(np.asarray(a).astype(BF16NP))


# ===================================================================
# Launch A: u = silu(im2col(x) @ W_fused + b_fused)
# ===================================================================
def build_launch_a(loop_reps=0):
    nc = bacc.Bacc("TRN2", target_bir_lowering=False, debug=False)
    xT = nc.dram_tensor("xT", [D, L + 2], BF16, kind="ExternalInput")
    wf = nc.dram_tensor("wf", [KT_X, 128, EL], BF16, kind="ExternalInput")
    bfu = nc.dram_tensor("bfu", [128, FT], F32, kind="ExternalInput")
    u_out = nc.dram_tensor("u_out", [EL, L], BF16, kind="ExternalOutput")

    with TileContext(nc) as tc:
        if loop_reps:
            tc.race_detector_enabled = False
        with (
            tc.tile_pool(name="big", bufs=1) as big,
            tc.tile_pool(name="work", bufs=4) as work,
            tc.tile_pool(name="ps", bufs=8, space="PSUM") as ps,
        ):
            xsb = big.tile([128, 8 * (L + 2)], BF16, tag="xsb")
            for j in range(8):
                nc.sync.dma_start(
                    xsb[:, j * (L + 2):(j + 1) * (L + 2)],
                    xT[j * 128:(j + 1) * 128, :])
            wfsb = big.tile([128, KT_X * EL], BF16, tag="wfsb")
            for kt in range(KT_X):
                nc.sync.dma_start(wfsb[:, kt * EL:(kt + 1) * EL], wf[kt, :, :])
            bfu_t = big.tile([128, FT], F32, tag="bfu")
            nc.sync.dma_start(bfu_t[:, :], bfu[:, :])

            def body(_=None, unroll=None):
                for ft in range(FT):
                    for tcI in range(TC):
                        t0 = tcI * 512
                        pt = ps.tile([128, 512], F32, tag="acc")
                        for kt in range(KT_X):
                            kap, j = divmod(kt, 8)
                            rhs = xsb[:, j * (L + 2) + t0 + kap:
                                      j * (L + 2) + t0 + kap + 512]
                            lhsT = wfsb[:, kt * EL + ft * 128:
                                        kt * EL + (ft + 1) * 128]
                            nc.tensor.matmul(pt[:, :], lhsT, rhs,
                                             start=(kt == 0),
                                             stop=(kt == KT_X - 1))
                        ut = work.tile([128, 512], BF16, tag="u")
                        nc.scalar.activation(ut[:, :], pt[:, :], AF.Silu,
                                             bias=bfu_t[:, ft:ft + 1])
                        nc.sync.dma_start(
                            u_out[ft * 128:(ft + 1) * 128, t0:t0 + 512],
                            ut[:, :])

            if loop_reps:
                with tc.For_i(0, loop_reps, 1) as _i:
                    body()
            else:
                body()
    nc.compile()
    return nc


# ===================================================================
# Launch B: projections + per-state scan + out-proj partial
# ===================================================================
def build_launch_b(loop_reps=0, stages=(1, 2, 3)):
    nc = bacc.Bacc("TRN2", target_bir_lowering=False, debug=False)
    u_all = nc.dram_tensor("u_all", [KT_U, 128, L], BF16, kind="ExternalInput")
    dtw = nc.dram_tensor("dtw", [KT_U, 128, EL], BF16, kind="ExternalInput")
    wbc = nc.dram_tensor("wbc", [KT_U, 128, 2 * N], BF16,
                         kind="ExternalInput")
    dtb = nc.dram_tensor("dtb", [128, FT], F32, kind="ExternalInput")
    bcb = nc.dram_tensor("bcb", [2 * N, 1], F32, kind="ExternalInput")
    acoln = nc.dram_tensor("acoln", [128, FT * N], F32, kind="ExternalInput")
    ident = nc.dram_tensor("ident", [128, 128], BF16, kind="ExternalInput")
    dsk = nc.dram_tensor("dsk", [128, FT], F32, kind="ExternalInput")
    ow = nc.dram_tensor("ow", [FT, 128, 8 * 128], BF16, kind="ExternalInput")
    u_own_in = nc.dram_tensor("u_own", [FT, 128, L], BF16,
                              kind="ExternalInput")
    yp = nc.dram_tensor("yp", [D, L], BF16, kind="ExternalOutput")
    bc_dram = nc.dram_tensor("bc_scratch", [2 * N, L], BF16, kind="Internal")

    with TileContext(nc) as tc:
        if loop_reps:
            tc.race_detector_enabled = False
        with (
            tc.tile_pool(name="big", bufs=1) as big,
            tc.tile_pool(name="w1", bufs=1) as w1,
            tc.tile_pool(name="bb", bufs=2) as bbp,
            tc.tile_pool(name="w2", bufs=2) as w2,
            tc.tile_pool(name="ut", bufs=4) as utp,
            tc.tile_pool(name="sm", bufs=2) as sm,
            tc.tile_pool(name="yt", bufs=1) as ytp,
            tc.tile_pool(name="ps", bufs=1, space="PSUM") as ps,
        ):
            # ---- resident weights/constants ----
            dtw_t = big.tile([128, KT_U * EL], BF16, tag="dtw")
            wbc_t = big.tile([128, KT_U * 2 * N], BF16, tag="wbc")
            for kt in range(KT_U):
                nc.sync.dma_start(dtw_t[:, kt * EL:(kt + 1) * EL],
                                  dtw[kt, :, :])
                nc.sync.dma_start(wbc_t[:, kt * 2 * N:(kt + 1) * 2 * N],
                                  wbc[kt, :, :])
            ow_t = big.tile([128, FT * 8 * 128], BF16, tag="ow")
            for ft in range(FT):
                nc.sync.dma_start(ow_t[:, ft * 1024:(ft + 1) * 1024],
                                  ow[ft, :, :])
            dtb_t = big.tile([128, FT], F32, tag="dtb")
            bcb_t = big.tile([2 * N, 1], F32, tag="bcb")
            acoln_t = big.tile([128, FT * N], F32, tag="acoln")
            dsk_t = big.tile([128, FT], F32, tag="dsk")
            ident_t = big.tile([128, 128], BF16, tag="ident")
            nc.sync.dma_start(dtb_t[:, :], dtb[:, :])
            nc.sync.dma_start(bcb_t[:, :], bcb[:, :])
            nc.sync.dma_start(acoln_t[:, :], acoln[:, :])
            nc.sync.dma_start(dsk_t[:, :], dsk[:, :])
            nc.sync.dma_start(ident_t[:, :], ident[:, :])

            # ---- resident activations ----
            qexp_t = big.tile([128, FT * L], BF16, tag="qexp")
            y_t = big.tile([128, FT * L], BF16, tag="y")

            def body(_=None, unroll=None):
                # ---- stage 1: dt/B/C projections ----
                for tcI in range(TC if 1 in stages else 0):
                    t0 = tcI * 512
                    psD = [ps.tile([128, 512], F32, tag=f"b{f}",
                                   name=f"psD{f}") for f in range(FT)]
                    psBC_full = ps.tile([128, 512], F32, tag="b4",
                                        name="psBC")
                    psBC = psBC_full[0:2 * N, :]
                    for kt in range(KT_U):
                        utile = utp.tile([128, 512], BF16, tag="u_in")
                        nc.sync.dma_start(utile[:, :],
                                          u_all[kt, :, t0:t0 + 512])
                        for ft in range(FT):
                            nc.tensor.matmul(
                                psD[ft][:, :],
                                dtw_t[:, kt * EL + ft * 128:
                                      kt * EL + (ft + 1) * 128],
                                utile[:, :], start=(kt == 0),
                                stop=(kt == KT_U - 1))
                        nc.tensor.matmul(
                            psBC,
                            wbc_t[:, kt * 2 * N:(kt + 1) * 2 * N],
                            utile[:, :], start=(kt == 0),
                            stop=(kt == KT_U - 1))
                    for ft in range(FT):
                        nc.scalar.activation(
                            qexp_t[:, ft * L + t0: ft * L + t0 + 512],
                            psD[ft][:, :], AF.Exp,
                            bias=dtb_t[:, ft:ft + 1])
                    bc_st = sm.tile([2 * N, 512], BF16, tag="bc_st")
                    nc.scalar.activation(bc_st[:, :], psBC,
                                         AF.Identity, bias=bcb_t[:, 0:1])
                    nc.sync.dma_start(bc_dram[:, t0:t0 + 512], bc_st[:, :])

                # ---- stage 2: per-state scans (ACT software-pipelined
                # one iteration ahead so psY copies never stall the DVE) ----
                nft = FT if 2 in stages else 0
                dsl = [None] * FT
                dAs = {}

                def emit_ln(ft):
                    d = w1.tile([128, L], BF16, tag=f"dlt{ft % 2}")
                    nc.scalar.activation(d[:, :],
                                         qexp_t[:, ft * L:(ft + 1) * L],
                                         AF.Ln, bias=1.0)
                    dsl[ft] = d

                def emit_da(ft, n):
                    dA = w2.tile([128, L], BF16, tag="dA")
                    nc.scalar.activation(
                        dA[:, :], dsl[ft][:, :], AF.Exp,
                        scale=acoln_t[:, ft * N + n: ft * N + n + 1])
                    dAs[(ft, n)] = dA

                if nft:
                    emit_ln(0)
                    emit_da(0, 0)
                for ft in range(nft):
                    uo = w1.tile([128, L], BF16, tag="h")
                    nc.sync.dma_start(uo[:, :], u_own_in[ft, :, :])
                    wloc = w1.tile([128, L], BF16, tag="wloc")
                    nc.vector.tensor_tensor(wloc[:, :], dsl[ft][:, :],
                                            uo[:, :], AL.mult)
                    psY = [ps.tile([128, 512], F32, tag=f"b{i}",
                                   name=f"psY{i}") for i in range(8)]
                    for n in range(N):
                        Bb = bbp.tile([128, L], BF16, tag="Bb")
                        nc.sync.dma_start(
                            Bb[:, :],
                            bc_dram[n:n + 1, :].partition_broadcast(128))
                        Cb = bbp.tile([128, L], BF16, tag="Cb")
                        nc.sync.dma_start(
                            Cb[:, :],
                            bc_dram[N + n:N + n + 1, :].partition_broadcast(128))
                        # emit next iteration's ACT work ahead of this one's
                        if n == N - 1 and ft + 1 < nft:
                            emit_ln(ft + 1)
                        if n + 1 < N:
                            emit_da(ft, n + 1)
                        elif ft + 1 < nft:
                            emit_da(ft + 1, 0)
                        dA = dAs.pop((ft, n))
                        X = w2.tile([128, L], BF16, tag="Xc")
                        nc.vector.tensor_tensor(X[:, :], wloc[:, :], Bb[:, :],
                                                AL.mult)
                        h = w1.tile([128, L], BF16, tag="h")
                        nc.vector.tensor_tensor_scan(h[:, :], dA[:, :],
                                                     X[:, :], 0.0,
                                                     AL.mult, AL.add)
                        ch = w2.tile([128, L], BF16, tag="Xc")
                        nc.vector.tensor_tensor(ch[:, :], h[:, :], Cb[:, :],
                                                AL.mult)
                        for tcI in range(TC):
                            nc.tensor.matmul(
                                psY[tcI][:, :], ident_t[:, :],
                                ch[:, tcI * 512:(tcI + 1) * 512],
                                start=(n == 0), stop=(n == N - 1))
                    for tcI in range(TC):
                        nc.scalar.activation(
                            y_t[:, ft * L + tcI * 512:
                                ft * L + tcI * 512 + 512],
                            psY[tcI][:, :], AF.Copy)

                # ---- stage 3: y_total & out-proj ----
                for tcI in range(TC if 3 in stages else 0):
                    t0 = tcI * 512
                    yt = [None] * FT
                    for ft in range(FT):
                        uo3 = sm.tile([128, 512], BF16, tag="uo3")
                        nc.sync.dma_start(uo3[:, :],
                                          u_own_in[ft, :, t0:t0 + 512])
                        ytf = ytp.tile([128, 512], BF16, tag=f"yt{ft}")
                        nc.vector.scalar_tensor_tensor(
                            ytf[:, :], uo3[:, :], dsk_t[:, ft:ft + 1],
                            y_t[:, ft * L + t0: ft * L + t0 + 512],
                            AL.mult, AL.add)
                        yt[ft] = ytf
                    for mt in range(8):
                        psO = ps.tile([128, 512], F32, tag=f"b{5 + mt % 3}",
                                      name="psO")
                        for ft in range(FT):
                            nc.tensor.matmul(
                                psO[:, :],
                                ow_t[:, ft * 1024 + mt * 128:
                                     ft * 1024 + (mt + 1) * 128],
                                yt[ft][:, :], start=(ft == 0),
                                stop=(ft == FT - 1))
                        ot = sm.tile([128, 512], BF16, tag="ot")
                        nc.scalar.activation(ot[:, :], psO[:, :], AF.Copy)
                        nc.sync.dma_start(
                            yp[mt * 128:(mt + 1) * 128, t0:t0 + 512],
                            ot[:, :])

            if loop_reps:
                with tc.For_i(0, loop_reps, 1) as _i:
                    body()
            else:
                body()
    nc.compile()
    return nc


# ===================================================================
# Host-side weight preparation
# ===================================================================
def prepare(inputs):
    x = np.asarray(inputs["x"], np.float32)
    conv_w = np.asarray(inputs["conv_w"], np.float32)
    conv_b = np.asarray(inputs["conv_b"], np.float32)
    in_w = np.asarray(inputs["in_w"], np.float32)
    in_b = np.asarray(inputs["in_b"], np.float32)
    A_log = np.asarray(inputs["A_log"], np.float32)
    Dskip = np.asarray(inputs["Dskip"], np.float32)
    dt_w = np.asarray(inputs["dt_w"], np.float32)
    dt_b = np.asarray(inputs["dt_b"], np.float32)
    Bp_w = np.asarray(inputs["Bp_w"], np.float32)
    Bp_b = np.asarray(inputs["Bp_b"], np.float32)
    Cp_w = np.asarray(inputs["Cp_w"], np.float32)
    Cp_b = np.asarray(inputs["Cp_b"], np.float32)
    out_w = np.asarray(inputs["out_w"], np.float32)
    out_b = np.asarray(inputs["out_b"], np.float32)

    # fused conv+in_proj: Wc[f,d,k] = sum_e in_w[f,e] conv_w[e,d,k]
    Wf = (in_w @ conv_w.reshape(E, D * K)).reshape(E, D, K)
    Wf_knl = Wf.transpose(2, 1, 0).reshape(K * D, E)   # [(kap,d), f]
    b_fused = in_w @ conv_b + in_b                      # [E]

    A = -np.exp(A_log)                                  # [E, N]

    prep = {"A": A}
    # per-batch xT padded
    prep["xT"] = []
    for b in range(B):
        xt = np.zeros((D, L + 2), np.float32)
        xt[:, 1:L + 1] = x[b].T
        prep["xT"].append(_bf16(xt))

    # per-shard tensors
    prep["wf"], prep["bfu"] = [], []
    prep["dtw"], prep["dtb"] = [], []
    prep["acoln"], prep["dskc"], prep["owk"] = [], [], []
    for s in range(TPG):
        Fc = slice(s * EL, (s + 1) * EL)
        prep["wf"].append(_bf16(Wf_knl[:, Fc].reshape(KT_X, 128, EL)))
        prep["bfu"].append(
            np.ascontiguousarray(b_fused[Fc].reshape(FT, 128).T,
                                 dtype=np.float32))
        prep["dtw"].append(_bf16(dt_w[Fc, :].T.reshape(KT_U, 128, EL)))
        prep["dtb"].append(
            np.ascontiguousarray(dt_b[Fc].reshape(FT, 128).T,
                                 dtype=np.float32))
        # acoln[p, ft*N+n] = A[s*EL + ft*128 + p, n]  (channel-major scale)
        A_sh = A[Fc].reshape(FT, 128, N)
        prep["acoln"].append(
            np.ascontiguousarray(A_sh.transpose(1, 0, 2).reshape(128, FT * N),
                                 dtype=np.float32))
        prep["dskc"].append(
            np.ascontiguousarray(Dskip[Fc].reshape(FT, 128).T,
                                 dtype=np.float32))
        # out-proj lhsT: ow[ft][p, mt*128+m] = out_w[mt*128+m, s*512+ft*128+p]
        owk = np.empty((FT, 128, 8 * 128), np.float32)
        for ft in range(FT):
            owk[ft] = out_w[:, s * EL + ft * 128:s * EL + (ft + 1) * 128].T
        prep["owk"].append(_bf16(owk))

    # joint B/C projection lhsT (replicated across cores): [KT_U, 128, 32]
    BC = np.vstack([Bp_w, Cp_w])                        # [32, E]
    prep["wbc_g"] = _bf16(BC.T.reshape(KT_U, 128, 2 * N))
    prep["bcb_g"] = np.ascontiguousarray(
        np.concatenate([Bp_b, Cp_b]).reshape(2 * N, 1), dtype=np.float32)
    prep["ident_g"] = _bf16(np.eye(128, dtype=np.float32))
    prep["out_b"] = out_b
    return prep


# ===================================================================
# Orchestration
# ===================================================================
_CACHE = {}


def _get_kernels(loop_reps=0):
    key = ("k", loop_reps)
    if key not in _CACHE:
        _CACHE[key] = (build_launch_a(loop_reps), build_launch_b(loop_reps))
    return _CACHE[key]


def run_launch_a(nca, prep, **kw):
    in_maps = []
    for c in range(NCORES):
        b, s = divmod(c, TPG)
        in_maps.append(dict(xT=prep["xT"][b], wf=prep["wf"][s],
                            bfu=prep["bfu"][s]))
    res = bass_utils.run_bass_kernel_spmd(nca, in_maps,
                                          core_ids=list(range(NCORES)), **kw)
    return [r["u_out"] for r in res.results]


def run_launch_b(ncb, prep, u_full, **kw):
    in_maps = []
    for c in range(NCORES):
        b, s = divmod(c, TPG)
        ub = u_full[b]                      # [E, L] bf16
        in_maps.append(dict(
            u_all=np.ascontiguousarray(ub.reshape(KT_U, 128, L)),
            u_own=np.ascontiguousarray(
                ub[s * EL:(s + 1) * EL].reshape(FT, 128, L)),
            dtw=prep["dtw"][s], wbc=prep["wbc_g"], dtb=prep["dtb"][s],
            bcb=prep["bcb_g"], acoln=prep["acoln"][s], ident=prep["ident_g"],
            dsk=prep["dskc"][s], ow=prep["owk"][s],
        ))
    res = bass_utils.run_bass_kernel_spmd(ncb, in_maps,
                                          core_ids=list(range(NCORES)), **kw)
    return [r["yp"] for r in res.results]


def kernel(**inputs):
    prep = prepare(inputs)
    nca, ncb = _get_kernels(_LOOP_REPS)
    u_shards = run_launch_a(nca, prep)          # 8 x [EL, L] bf16
    u_full = []
    for b in range(B):
        u_full.append(np.concatenate(u_shards[b * TPG:(b + 1) * TPG], axis=0))
    yps = run_launch_b(ncb, prep, u_full)       # 8 x [D, L] fp32
    out = np.empty((B, L, D), np.float32)
    for b in range(B):
        acc = np.asarray(yps[b * TPG], np.float32)
        for s in range(1, TPG):
            acc = acc + np.asarray(yps[b * TPG + s], np.float32)
        out[b] = acc.T + prep["out_b"][None, :]
    return out
